# revision 7
# baseline (speedup 1.0000x reference)
"""GPT-2 style causal attention block (B=4, S=2048, E=1024, H=16, D=64) on
8 TRN2 NeuronCores.

Sharding: batch(4) x head-half(2) -> 8 cores, zero on-device communication.
Core c handles batch b=c//2 and heads h0=(c%2)*8 .. h0+7. Each core computes
its qkv column block, attention for its 8 heads, and a partial c_proj
(its 512 rows of w_proj). The two partial outputs per batch are summed on the
host during unshard (b_proj is given only to the even core of each pair).

On-device layout (per core, everything transposed so softmax reduction lands
on the PE via a ones-column in the V weights):
  X^T  [E, S]   via PE transposes of the DMA'd X tiles
  qkv^T = W^T-stationary matmuls -> Q^T,K^T [1024, S];  V computed in [S, c]
  orientation with an interleaved ones-column (rowsum comes out of attn@V)
  scores^T[k, q] per head; exp on ACT (scale=1/8 folded in); causal handled by
  computing only k<=q tiles + one [128,128] triangular mask on diagonal tiles
  attn@V accumulates out^T[1+64, q] in PSUM; row 0 = softmax denominator
  c_proj from A^T [512, S] with W_proj rows -> y [S, E] partial.

Compute dtype bf16 (PE), accumulation f32 (PSUM).
"""

import re

import numpy as np

import concourse.bass as bass
import concourse.mybir as mybir
import concourse.tile as tile
from concourse import bacc
from concourse.bass_utils import run_bass_kernel_spmd
from concourse.masks import make_identity
from concourse.vector_clock import ScopedClock

F32 = mybir.dt.float32
BF16 = mybir.dt.bfloat16
AF = mybir.ActivationFunctionType

S = 2048          # sequence length (per batch)
E = 1024          # embedding dim
HL = 8            # heads per core
D = 64            # head dim
TT = S // 128     # 16 token tiles
ET = E // 128     # 8 embedding tiles
NCH = S // 512    # 4 q-chunks of 512
VW = HL * (D + 1)  # 520: V block width with interleaved ones-columns


def _install_drain_fix():
    """walrus in this container rejects the Tile kernel-tail Drain when it
    carries all semaphore waits on one instruction ("Too many sync wait
    commands"). Emit one wait_ge per semaphore, then a bare drain."""
    if getattr(tile.TileContext, "_drain_fix_installed", False):
        return

    def _split_drain_and_barrier(self, tick_clock, wait_clock):
        nc = self.nc
        probe = mybir.InstDrain(
            name="probe-drain", engine=mybir.EngineType.SP, ins=[], outs=[]
        )
        wait_clock.add_sem_waits(probe, ScopedClock({None: tick_clock.global_clock}))
        waits = re.findall(r"wait:S\[([A-Za-z0-9_]+)\]>=(\d+)", probe.concise())
        handles = {h.name: h for h in self.sems.allocated().values()}
        for name, val in waits:
            nc.sync.wait_ge(handles[name], int(val))
        nc.sync.drain()
        nc.all_engine_barrier()
        popped = nc._tile_sem_poison_stack.pop()
        assert popped is self._sem_poison
        nc.clear_and_free_semaphores(list(self.sems.allocated().values()))
        nc.all_engine_barrier()

    tile.TileContext._drain_and_barrier = _split_drain_and_barrier
    tile.TileContext._drain_fix_installed = True


def _emit(nc, tc, ctx):
    x_d = nc.declare_dram_parameter("x", [S, E], F32, isOutput=False)
    wqk_d = nc.declare_dram_parameter("wqk", [E, 1024], F32, isOutput=False)
    wva_d = nc.declare_dram_parameter("wva", [E, VW], F32, isOutput=False)
    wp_d = nc.declare_dram_parameter("wp", [512, E], F32, isOutput=False)
    bqk_d = nc.declare_dram_parameter("bqk", [8, 128, 1], F32, isOutput=False)
    bva_d = nc.declare_dram_parameter("bva", [1, VW], F32, isOutput=False)
    bp_d = nc.declare_dram_parameter("bp", [1, E], F32, isOutput=False)
    mask_d = nc.declare_dram_parameter("mask", [128, 128], F32, isOutput=False)
    out_d = nc.declare_dram_parameter("out", [S, E], F32, isOutput=True)

    consts = ctx.enter_context(tc.tile_pool(name="consts", bufs=1))
    statics = ctx.enter_context(tc.tile_pool(name="statics", bufs=1))
    stage = ctx.enter_context(tc.tile_pool(name="stage", bufs=4))
    ptp = ctx.enter_context(tc.tile_pool(name="ptp", bufs=4))
    rp = ctx.enter_context(tc.tile_pool(name="rp", bufs=4))
    yp = ctx.enter_context(tc.tile_pool(name="yp", bufs=3))
    psA = ctx.enter_context(tc.tile_pool(name="psA", bufs=2, space="PSUM"))
    psV = ctx.enter_context(tc.tile_pool(name="psV", bufs=1, space="PSUM"))

    # ---- constants ----
    ident = consts.tile([128, 128], F32)
    make_identity(nc, ident)
    mask_f = consts.tile([128, 128], F32)
    nc.sync.dma_start(out=mask_f, in_=mask_d[:])
    mask_b = consts.tile([128, 128], BF16)
    nc.vector.tensor_copy(mask_b, mask_f)
    bqk_sb = consts.tile([128, 8], F32)
    for m in range(8):
        nc.sync.dma_start(out=bqk_sb[:, m : m + 1], in_=bqk_d[m])
    bva_st = consts.tile([1, VW], F32)
    nc.sync.dma_start(out=bva_st, in_=bva_d[:])
    bva_bc = consts.tile([128, VW], F32)
    nc.gpsimd.partition_broadcast(out_ap=bva_bc[:], in_ap=bva_st[:])
    bp_st = consts.tile([1, E], F32)
    nc.sync.dma_start(out=bp_st, in_=bp_d[:])
    bp_bc = consts.tile([128, E], F32)
    nc.gpsimd.partition_broadcast(out_ap=bp_bc[:], in_ap=bp_st[:])

    # ---- weights: DMA f32 staging -> bf16 statics ----
    wqk_sb = statics.tile([128, ET, 1024], BF16)
    wva_sb = statics.tile([128, ET, VW], BF16)
    wp_sb = statics.tile([128, 4, E], BF16)
    for et in range(ET):
        st = stage.tile([128, 1024], F32, tag="stage")
        nc.sync.dma_start(out=st, in_=wqk_d[et * 128 : (et + 1) * 128, :])
        nc.vector.tensor_copy(wqk_sb[:, et, :], st)
    for et in range(ET):
        st = stage.tile([128, 1024], F32, tag="stage")
        nc.sync.dma_start(out=st[:, :VW], in_=wva_d[et * 128 : (et + 1) * 128, :])
        nc.vector.tensor_copy(wva_sb[:, et, :], st[:, :VW])
    for ct in range(4):
        st = stage.tile([128, 1024], F32, tag="stage")
        nc.sync.dma_start(out=st, in_=wp_d[ct * 128 : (ct + 1) * 128, :])
        nc.vector.tensor_copy(wp_sb[:, ct, :], st)

    # ---- X^T via PE transpose ----
    xt_sb = statics.tile([128, ET, S], BF16)
    for i in range(TT):
        xst = stage.tile([128, E], F32, tag="stage")
        nc.sync.dma_start(out=xst, in_=x_d[i * 128 : (i + 1) * 128, :])
        for et in range(ET):
            pst = psA.tile([128, 128], F32, tag="s")
            nc.tensor.transpose(pst, xst[:, et * 128 : (et + 1) * 128], ident)
            nc.scalar.copy(out=xt_sb[:, et, i * 128 : (i + 1) * 128], in_=pst)

    # ---- qkv: Q^T,K^T [1024, S] (W stationary) ----
    qkt_sb = statics.tile([128, 8, S], BF16)
    for m in range(8):
        for tch in range(NCH):
            pqk = psA.tile([128, 512], F32, tag="qk")
            for et in range(ET):
                nc.tensor.matmul(
                    pqk,
                    wqk_sb[:, et, m * 128 : (m + 1) * 128],
                    xt_sb[:, et, tch * 512 : (tch + 1) * 512],
                    start=(et == 0),
                    stop=(et == ET - 1),
                )
            nc.scalar.activation(
                out=qkt_sb[:, m, tch * 512 : (tch + 1) * 512],
                in_=pqk,
                func=AF.Identity,
                bias=bqk_sb[:, m : m + 1],
                scale=1.0,
            )

    # ---- V (+ interleaved ones cols) in [S, VW] (X^T stationary) ----
    va_sb = statics.tile([128, TT, VW], BF16)
    for i in range(TT):
        pv = psV.tile([128, VW], F32, tag="v")
        for et in range(ET):
            nc.tensor.matmul(
                pv[:, 0:512],
                xt_sb[:, et, i * 128 : (i + 1) * 128],
                wva_sb[:, et, 0:512],
                start=(et == 0),
                stop=(et == ET - 1),
            )
        for et in range(ET):
            nc.tensor.matmul(
                pv[:, 512:VW],
                xt_sb[:, et, i * 128 : (i + 1) * 128],
                wva_sb[:, et, 512:VW],
                start=(et == 0),
                stop=(et == ET - 1),
            )
        nc.vector.tensor_add(va_sb[:, i, :], pv, bva_bc)

    # ---- attention per head ----
    at_sb = statics.tile([128, 4, S], BF16)  # A^T: rows c=h*64+d, cols t
    for h in range(HL):
        po = (h % 2) * 64  # partition offset of this head inside its 128-tile
        qm, km = h // 2, 4 + h // 2
        for j in range(NCH):
            pso = psA.tile([65, 512], F32, tag="o")
            nkt = 4 * (j + 1)
            for kt in range(nkt):
                p = kt - 4 * j
                off = max(0, p * 128)
                n = 512 - off
                pss = psA.tile([128, 512], F32, tag="s")
                nc.tensor.matmul(
                    pss[:, 0:n],
                    qkt_sb[po : po + 64, km, kt * 128 : (kt + 1) * 128],
                    qkt_sb[po : po + 64, qm, j * 512 + off : (j + 1) * 512],
                    start=True,
                    stop=True,
                )
                pt = ptp.tile([128, 512], BF16, tag="pt")
                nc.scalar.activation(
                    out=pt[:, 0:n], in_=pss[:, 0:n], func=AF.Exp, scale=0.125
                )
                if p >= 0:
                    nc.vector.tensor_mul(pt[:, 0:128], pt[:, 0:128], mask_b)
                nc.tensor.matmul(
                    pso[:, off:512],
                    va_sb[:, kt, h * 65 : (h + 1) * 65],
                    pt[:, 0:n],
                    start=(kt == 0),
                    stop=(kt == nkt - 1),
                )
            rinv = rp.tile([1, 512], F32, tag="ri")
            nc.vector.reciprocal(out=rinv, in_=pso[64:65, :])
            rbc = rp.tile([64, 512], F32, tag="rb")
            nc.gpsimd.partition_broadcast(out_ap=rbc[:], in_ap=rinv[:])
            nc.vector.tensor_mul(
                at_sb[po : po + 64, h // 2, j * 512 : (j + 1) * 512],
                pso[0:64, :],
                rbc,
            )

    # ---- c_proj partial: y[S, E] ----
    for i in range(TT):
        ysb = yp.tile([128, E], F32, tag="y")
        for ech in range(2):
            py = psA.tile([128, 512], F32, tag="qk")
            for ct in range(4):
                nc.tensor.matmul(
                    py,
                    at_sb[:, ct, i * 128 : (i + 1) * 128],
                    wp_sb[:, ct, ech * 512 : (ech + 1) * 512],
                    start=(ct == 0),
                    stop=(ct == 3),
                )
            nc.vector.tensor_add(
                ysb[:, ech * 512 : (ech + 1) * 512],
                py,
                bp_bc[:, ech * 512 : (ech + 1) * 512],
            )
        nc.sync.dma_start(out=out_d[i * 128 : (i + 1) * 128, :], in_=ysb)


def build_nc():
    _install_drain_fix()
    from contextlib import ExitStack

    nc = bacc.Bacc()
    with ExitStack() as ctx:
        tc = ctx.enter_context(tile.TileContext(nc))
        _emit(nc, tc, ctx)
    nc.finalize()  # Bacc: alloc_regs + insert_library_loads happen here
    return nc


def make_in_maps(inputs, w_attn, b_attn, w_proj, b_proj):
    """Build the 8 per-core input dicts from the full tensors."""
    x = np.ascontiguousarray(np.asarray(inputs, dtype=np.float32))
    w_attn = np.asarray(w_attn, dtype=np.float32)
    b_attn = np.asarray(b_attn, dtype=np.float32)
    w_proj = np.asarray(w_proj, dtype=np.float32)
    b_proj = np.asarray(b_proj, dtype=np.float32)

    mask = (np.arange(128)[None, :] >= np.arange(128)[:, None]).astype(np.float32)
    in_maps = []
    for c in range(8):
        b, half = c // 2, c % 2
        h0 = half * 8
        cols = np.arange(h0 * 64, h0 * 64 + 512)
        wqk = np.ascontiguousarray(
            np.concatenate([w_attn[:, cols], w_attn[:, 1024 + cols]], axis=1)
        )
        bqk = np.concatenate([b_attn[cols], b_attn[1024 + cols]]).reshape(8, 128, 1)
        wva = np.zeros((E, VW), dtype=np.float32)
        bva = np.zeros((1, VW), dtype=np.float32)
        for h in range(HL):
            src = 2048 + (h0 + h) * 64
            wva[:, h * 65 : h * 65 + 64] = w_attn[:, src : src + 64]
            bva[0, h * 65 + 64] = 1.0
            bva[0, h * 65 : h * 65 + 64] = b_attn[src : src + 64]
        wp = np.ascontiguousarray(w_proj[h0 * 64 : h0 * 64 + 512, :])
        bp = (b_proj if half == 0 else np.zeros_like(b_proj)).reshape(1, E)
        in_maps.append(
            {
                "x": np.ascontiguousarray(x[b]),
                "wqk": wqk,
                "wva": wva,
                "wp": wp,
                "bqk": np.ascontiguousarray(bqk.astype(np.float32)),
                "bva": bva,
                "bp": np.ascontiguousarray(bp.astype(np.float32)),
                "mask": mask,
            }
        )
    return in_maps


_CACHE = {}


def kernel(**inputs):
    nc = _CACHE.get("nc")
    if nc is None:
        nc = _CACHE["nc"] = build_nc()
    in_maps = make_in_maps(
        inputs["inputs"],
        inputs["w_attn"],
        inputs["b_attn"],
        inputs["w_proj"],
        inputs["b_proj"],
    )
    res = run_bass_kernel_spmd(nc, in_maps, core_ids=list(range(8)))
    out = np.zeros((4, S, E), dtype=np.float32)
    for b in range(4):
        out[b] = res.results[2 * b]["out"] + res.results[2 * b + 1]["out"]
    return out


# revision 35
# speedup vs baseline: 1.2592x; 1.2592x over previous
"""GPT-2 style causal attention block (B=4, S=2048, E=1024, H=16, D=64) on
8 TRN2 NeuronCores.

Sharding: batch(4) x head-half(2) -> 8 cores, zero on-device communication.
Core c handles batch b=c//2 and heads h0=(c%2)*8 .. h0+7. Each core computes
its qkv column block, attention for its 8 heads, and a partial c_proj
(its 512 rows of w_proj). The two partial outputs per batch are summed on the
host during unshard (b_proj is given only to the even core of each pair).

On-device layout (per core, everything transposed so the softmax reduction
lands on the PE via a ones-column appended to each head's V block):
  X^T  [E, S]    bf16, via PE transposes of DMA'd X tiles
  Q^T,K^T [1024, S]  W-stationary matmuls; V in [S, 520] ([64 d | 1] per head)
  scores^T[k, q] per head; exp on ACT (1/sqrt(D) folded into the act scale);
  causality by computing only k<=q tiles + one [128,128] triangular mask on
  diagonal 128-blocks
  attn@V accumulates out^T[64+1, q] in PSUM; row 64 = softmax denominator
  c_proj from A^T [512, S] with this core's W_proj rows -> partial y [S, E]

X / weights / mask are pre-converted to bf16 on the host (compute dtype);
accumulation is f32 in PSUM; output f32.
"""

import re

import ml_dtypes
import numpy as np

import concourse.bass as bass
import concourse.mybir as mybir
import concourse.tile as tile
from concourse import bacc
from concourse.bass_utils import run_bass_kernel_spmd
from concourse.masks import make_identity
from concourse.vector_clock import ScopedClock

F32 = mybir.dt.float32
BF16 = mybir.dt.bfloat16
BF16_NP = ml_dtypes.bfloat16
AF = mybir.ActivationFunctionType

S = 2048          # sequence length (per batch)
E = 1024          # embedding dim
HL = 8            # heads per core
D = 64            # head dim
TT = S // 128     # 16 token tiles
ET = E // 128     # 8 embedding tiles
NCH = S // 1024   # 2 q-chunks of 1024
VW = HL * (D + 1)  # 520: V block width with per-head ones-column


def _install_drain_fix():
    """walrus in this container rejects the Tile kernel-tail Drain when it
    carries all semaphore waits on one instruction ("Too many sync wait
    commands"). Emit one wait_ge per semaphore, then a bare drain."""
    if getattr(tile.TileContext, "_drain_fix_installed", False):
        return

    def _split_drain_and_barrier(self, tick_clock, wait_clock):
        nc = self.nc
        probe = mybir.InstDrain(
            name="probe-drain", engine=mybir.EngineType.SP, ins=[], outs=[]
        )
        wait_clock.add_sem_waits(probe, ScopedClock({None: tick_clock.global_clock}))
        waits = re.findall(r"wait:S\[([A-Za-z0-9_]+)\]>=(\d+)", probe.concise())
        handles = {h.name: h for h in self.sems.allocated().values()}
        for name, val in waits:
            nc.sync.wait_ge(handles[name], int(val))
        nc.sync.drain()
        nc.all_engine_barrier()
        popped = nc._tile_sem_poison_stack.pop()
        assert popped is self._sem_poison
        nc.clear_and_free_semaphores(list(self.sems.allocated().values()))
        nc.all_engine_barrier()

    tile.TileContext._drain_and_barrier = _split_drain_and_barrier
    tile.TileContext._drain_fix_installed = True


def _emit(nc, tc, ctx):
    xt_d = nc.declare_dram_parameter("xt", [E, S], BF16, isOutput=False)
    wqk_d = nc.declare_dram_parameter("wqk", [E, 1024], BF16, isOutput=False)
    wva_d = nc.declare_dram_parameter("wva", [E, VW], BF16, isOutput=False)
    wp_d = nc.declare_dram_parameter("wp", [512, E], BF16, isOutput=False)
    bqk_d = nc.declare_dram_parameter("bqk", [8, 128, 1], F32, isOutput=False)
    bva_d = nc.declare_dram_parameter("bva", [1, VW], F32, isOutput=False)
    bp_d = nc.declare_dram_parameter("bp", [1, E], F32, isOutput=False)
    mask_d = nc.declare_dram_parameter("mask", [128, 128], BF16, isOutput=False)
    out_d = nc.declare_dram_parameter("out", [S, E], F32, isOutput=True)

    consts = ctx.enter_context(tc.tile_pool(name="consts", bufs=1))
    statics = ctx.enter_context(tc.tile_pool(name="statics", bufs=1))
    stage = ctx.enter_context(tc.tile_pool(name="stage", bufs=3))
    ptp = ctx.enter_context(tc.tile_pool(name="ptp", bufs=5))
    rp = ctx.enter_context(tc.tile_pool(name="rp", bufs=2))
    yp = ctx.enter_context(tc.tile_pool(name="yp", bufs=3))
    # PSUM budget (8 banks): sc 2x[128,1024]=4, o 1x[65,1024]=2, qk 2x[128,512]=2
    psS = ctx.enter_context(tc.tile_pool(name="psS", bufs=2, space="PSUM"))
    psO = ctx.enter_context(tc.tile_pool(name="psO", bufs=1, space="PSUM"))
    psQ = ctx.enter_context(tc.tile_pool(name="psQ", bufs=2, space="PSUM"))

    # ---- front section: DMA order matters (the DMA engines are a single
    # serialized 360GB/s resource). X^T arrives host-pre-transposed; wqk
    # interleaves so qkv unlocks early; wp (needed last) at the end ----
    xt_sb = statics.tile([128, ET, S], BF16)
    wqk_sb = statics.tile([128, ET, 1024], BF16)
    wva_sb = statics.tile([128, ET, VW], BF16)
    wp_sb = statics.tile([128, 4, E], BF16)

    for et in range(ET):
        nc.sync.dma_start(
            out=xt_sb[:, et, :], in_=xt_d[et * 128 : (et + 1) * 128, :]
        )
        nc.gpsimd.dma_start(
            out=wqk_sb[:, et, :], in_=wqk_d[et * 128 : (et + 1) * 128, :]
        )
    for et in range(ET):
        nc.gpsimd.dma_start(
            out=wva_sb[:, et, :], in_=wva_d[et * 128 : (et + 1) * 128, :]
        )
    for ct in range(4):
        nc.gpsimd.dma_start(out=wp_sb[:, ct, :], in_=wp_d[ct * 128 : (ct + 1) * 128, :])

    mask_b = consts.tile([128, 128], BF16)
    nc.sync.dma_start(out=mask_b, in_=mask_d[:])
    bqk_sb = consts.tile([128, 8], F32)
    for m in range(8):
        nc.sync.dma_start(out=bqk_sb[:, m : m + 1], in_=bqk_d[m])
    bva_st = consts.tile([1, VW], F32)
    nc.sync.dma_start(out=bva_st, in_=bva_d[:])
    bva_bc = consts.tile([128, VW], F32)
    nc.gpsimd.partition_broadcast(out_ap=bva_bc[:], in_ap=bva_st[:])
    bp_st = consts.tile([1, E], F32)
    nc.sync.dma_start(out=bp_st, in_=bp_d[:])
    bp_bc = consts.tile([128, E], F32)
    nc.gpsimd.partition_broadcast(out_ap=bp_bc[:], in_ap=bp_st[:])

    # ---- qkv Q^T,K^T (W stationary) paired so head h's Q and K m-tiles
    # arrive together, interleaved with V tiles -> attention starts early ----
    qkt_sb = statics.tile([128, 8, S], BF16)
    va_sb = statics.tile([128, TT, VW], BF16)

    def emit_qk(m):
        for tch in range(4):
            pqk = psQ.tile([128, 512], F32, tag="qk")
            for et in range(ET):
                nc.tensor.matmul(
                    pqk,
                    wqk_sb[:, et, m * 128 : (m + 1) * 128],
                    xt_sb[:, et, tch * 512 : (tch + 1) * 512],
                    start=(et == 0),
                    stop=(et == ET - 1),
                )
            nc.vector.tensor_scalar_add(
                qkt_sb[:, m, tch * 512 : (tch + 1) * 512], pqk, bqk_sb[:, m : m + 1]
            )

    def emit_v(i):
        pv1 = psQ.tile([128, 512], F32, tag="qk")
        for et in range(ET):
            nc.tensor.matmul(
                pv1,
                xt_sb[:, et, i * 128 : (i + 1) * 128],
                wva_sb[:, et, 0:512],
                start=(et == 0),
                stop=(et == ET - 1),
            )
        nc.vector.tensor_add(va_sb[:, i, 0:512], pv1, bva_bc[:, 0:512])
        pv2 = psQ.tile([128, 8], F32, tag="qk")
        for et in range(ET):
            nc.tensor.matmul(
                pv2,
                xt_sb[:, et, i * 128 : (i + 1) * 128],
                wva_sb[:, et, 512:VW],
                start=(et == 0),
                stop=(et == ET - 1),
            )
        nc.vector.tensor_add(va_sb[:, i, 512:VW], pv2, bva_bc[:, 512:VW])

    for m in range(4):
        emit_qk(m)      # Q m-tile: heads 2m, 2m+1
        emit_qk(4 + m)  # K m-tile: heads 2m, 2m+1
        # V tiles 0-7 (all chunk-0 attention needs); 8-15 are deferred into
        # the chunk-0 head loop as PE filler for the ACT-bound stretch
        emit_v(2 * m)
        emit_v(2 * m + 1)

    # ---- attention (q-chunks of 1024), interleaved with c_proj halves ----
    at_sb = statics.tile([128, 4, S], BF16)  # A^T: rows c=h*64+d, cols t

    def segs(off):
        if off < 512:
            return [(off, 512), (512, 1024)]
        return [(off, 1024)]

    def emit_cproj(i):
        ysb = yp.tile([128, E], F32, tag="y")
        for ech in range(2):
            py = psQ.tile([128, 512], F32, tag="qk")
            for ct in range(4):
                nc.tensor.matmul(
                    py,
                    at_sb[:, ct, i * 128 : (i + 1) * 128],
                    wp_sb[:, ct, ech * 512 : (ech + 1) * 512],
                    start=(ct == 0),
                    stop=(ct == 3),
                )
            nc.vector.tensor_add(
                ysb[:, ech * 512 : (ech + 1) * 512],
                py,
                bp_bc[:, ech * 512 : (ech + 1) * 512],
            )
        nc.sync.dma_start(out=out_d[i * 128 : (i + 1) * 128, :], in_=ysb)

    for j in range(NCH):
        q0 = j * 1024
        nkt = 8 * (j + 1)
        # attn@V piece list per kt, with PSUM group flags: the sim (and HW
        # pending-zero) track groups per 2KB bank keyed by each matmul's
        # START byte — the first piece starting in a bank carries start=True
        # (marks the whole bank pending-zero), the last carries stop=True.
        av_pieces = []  # (kt, a, b)
        for kt in range(nkt):
            p = kt - 8 * j
            off = max(0, p * 128)
            for a, b in segs(off):
                av_pieces.append((kt, a, b))
        first_in_bank, last_in_bank = {}, {}
        for idx, (kt, a, b) in enumerate(av_pieces):
            bank = a // 512
            first_in_bank.setdefault(bank, idx)
            last_in_bank[bank] = idx
        starts = set(first_in_bank.values())
        stops = set(last_in_bank.values())

        for h in range(HL):
            po = (h % 2) * 64
            qm, km = h // 2, 4 + h // 2
            pso = psO.tile([65, 1024], F32, tag="o")
            # drain each pso bank to SBUF right after its last av write so
            # the single psO slot frees as early as possible
            osb = rp.tile([65, 1024], F32, tag="os")
            idx = 0
            for kt in range(nkt):
                p = kt - 8 * j
                off = max(0, p * 128)
                ps2 = psS.tile([128, 1024], F32, tag="sc")
                for a, b in segs(off):
                    nc.tensor.matmul(
                        ps2[:, a:b],
                        qkt_sb[po : po + 64, km, kt * 128 : (kt + 1) * 128],
                        qkt_sb[po : po + 64, qm, q0 + a : q0 + b],
                        start=True,
                        stop=True,
                    )
                pt = ptp.tile([128, 1024], BF16, tag="pt")
                nc.scalar.activation(
                    out=pt[:, off:1024], in_=ps2[:, off:1024], func=AF.Exp, scale=0.125
                )
                if p >= 0:
                    nc.vector.tensor_mul(
                        pt[:, off : off + 128], pt[:, off : off + 128], mask_b
                    )
                while idx < len(av_pieces) and av_pieces[idx][0] == kt:
                    _, a, b = av_pieces[idx]
                    nc.tensor.matmul(
                        pso[:, a:b],
                        va_sb[:, kt, h * 65 : (h + 1) * 65],
                        pt[:, a:b],
                        start=(idx in starts),
                        stop=(idx in stops),
                    )
                    if idx == last_in_bank[0]:
                        nc.vector.tensor_copy(osb[:, 0:512], pso[:, 0:512])
                    elif idx == last_in_bank[1]:
                        nc.vector.tensor_copy(osb[:, 512:1024], pso[:, 512:1024])
                    idx += 1
            rinv = rp.tile([1, 1024], F32, tag="ri")
            nc.vector.reciprocal(out=rinv, in_=osb[64:65, :])
            rbc = rp.tile([64, 1024], F32, tag="rb")
            nc.gpsimd.partition_broadcast(out_ap=rbc[:], in_ap=rinv[:])
            nc.vector.tensor_mul(
                at_sb[po : po + 64, h // 2, q0 : q0 + 1024], osb[0:64, :], rbc
            )
            if j == 0:
                # V tiles 8-15 (needed only by chunk 1) as PE filler while
                # chunk-0 attention is ACT(exp)-rate-bound
                emit_v(8 + h)
            else:
                # chunk-0 c_proj tiles as PE filler for chunk-1 attention
                emit_cproj(h)
    for i in range(8, 16):
        emit_cproj(i)


def build_nc():
    _install_drain_fix()
    from contextlib import ExitStack

    nc = bacc.Bacc()
    with ExitStack() as ctx:
        tc = ctx.enter_context(tile.TileContext(nc))
        _emit(nc, tc, ctx)
    nc.finalize()  # Bacc: alloc_regs + insert_library_loads happen here
    return nc


def make_in_maps(inputs, w_attn, b_attn, w_proj, b_proj):
    """Build the 8 per-core input dicts from the full tensors.
    X / weights / mask go down pre-converted to bf16 (the compute dtype)."""
    x = np.asarray(inputs, dtype=np.float32)
    w_attn = np.asarray(w_attn, dtype=np.float32)
    b_attn = np.asarray(b_attn, dtype=np.float32)
    w_proj = np.asarray(w_proj, dtype=np.float32)
    b_proj = np.asarray(b_proj, dtype=np.float32)

    mask = (np.arange(128)[None, :] >= np.arange(128)[:, None]).astype(BF16_NP)
    in_maps = []
    for c in range(8):
        b, half = c // 2, c % 2
        h0 = half * 8
        cols = np.arange(h0 * 64, h0 * 64 + 512)
        wqk = np.ascontiguousarray(
            np.concatenate([w_attn[:, cols], w_attn[:, 1024 + cols]], axis=1).astype(
                BF16_NP
            )
        )
        bqk = np.concatenate([b_attn[cols], b_attn[1024 + cols]]).reshape(8, 128, 1)
        wva = np.zeros((E, VW), dtype=np.float32)
        bva = np.zeros((1, VW), dtype=np.float32)
        for h in range(HL):
            src = 2048 + (h0 + h) * 64
            wva[:, h * 65 : h * 65 + 64] = w_attn[:, src : src + 64]
            bva[0, h * 65 + 64] = 1.0
            bva[0, h * 65 : h * 65 + 64] = b_attn[src : src + 64]
        wp = np.ascontiguousarray(w_proj[h0 * 64 : h0 * 64 + 512, :].astype(BF16_NP))
        bp = (b_proj if half == 0 else np.zeros_like(b_proj)).reshape(1, E)
        in_maps.append(
            {
                "xt": np.ascontiguousarray(x[b].T.astype(BF16_NP)),
                "wqk": wqk,
                "wva": np.ascontiguousarray(wva.astype(BF16_NP)),
                "wp": wp,
                "bqk": np.ascontiguousarray(bqk.astype(np.float32)),
                "bva": bva,
                "bp": np.ascontiguousarray(bp.astype(np.float32)),
                "mask": mask,
            }
        )
    return in_maps


_CACHE = {}


def kernel(**inputs):
    nc = _CACHE.get("nc")
    if nc is None:
        nc = _CACHE["nc"] = build_nc()
    in_maps = make_in_maps(
        inputs["inputs"],
        inputs["w_attn"],
        inputs["b_attn"],
        inputs["w_proj"],
        inputs["b_proj"],
    )
    res = run_bass_kernel_spmd(nc, in_maps, core_ids=list(range(8)))
    out = np.zeros((4, S, E), dtype=np.float32)
    for b in range(4):
        out[b] = res.results[2 * b]["out"] + res.results[2 * b + 1]["out"]
    return out
